# revision 13
# baseline (speedup 1.0000x reference)
"""Multi-head causal self-attention (D=768, H=12, S=4096) on 8 Trainium2 cores.

Sharding: 4 head-groups (3 heads each) x 2 interleaved query-sets.
Core c = 2*g + s owns head-group g (heads 3g..3g+2) and query 128-row
blocks s, s+2, s+4, ... (even/odd interleave balances the causal
triangle).  Every core runs the SAME program; per-core behaviour is
driven entirely by input data.  Each core produces a partial [2048, 768]
output (its heads pushed through its slice of Wo, all biases folded
in); the host sums the 4 group partials per query-set and re-interleaves
rows.

v3.1 key points:
  - QK^T matmuls are fp8e4 DoubleRow (half the PE columns): Q and K live
    in zero-padded [128, 2, *] fp8 tiles so all three heads share one
    stationary operand; zero regions kill cross-head terms.  The zero
    fills are region-exact DMAs so they never fence the evictions.
  - Q/K projections are fp8 DoubleRow with dual-rail (hi+lo) fp8 weights.
  - attention runs on 256-query tiles with all 3 heads' scores packed in
    ONE psum tile (2 banks, double-buffered bank-disjoint; PE-write +
    ACT-read on one bank is fatal), one exp instr per key block; psum =
    4 (scores) + 3 (pv) + 1 (projection) banks.
  - projection work is sliced into accumulation groups and fed into the
    attention m-loop just-in-time (deadline-paced), so the PE stream
    interleaves projections under the ACT-bound attention instead of
    bursting between groups.
  - normalize uses gpsimd partition_broadcast + a fused psum*bf16 DVE
    multiply; causal-mask multiplies run on GPSIMD; V/P/O stay bf16
    (fp8 there costs ~2.4% output noise).
"""

import numpy as np

D = 768
S = 4096
H = 12
HD = 64
NG = 4          # head groups
GH = 3          # heads per group
GD = GH * HD    # 192 dims per group
SL = S // 2     # local queries per core (2048)
P = 128
NC = D // P     # 6 contraction chunks
QG = 4          # projection chunks (512 q each)
QGS = 512
NT = 8          # attention query groups (256 q each)
QT = 256
NKB = S // P    # 32 key blocks
NQB = SL // P   # 16 local query tiles

DRPROJ = True   # Q/K projections via fp8 dual-rail DoubleRow
WSCALE = 16.0   # pow-2 scale keeping dual-rail fp8 weights in normal range

_CACHE = {}


def _build_program():
    import concourse.bacc as bacc
    import concourse.mybir as mybir
    import concourse.tile as tile
    from contextlib import ExitStack

    bf16 = mybir.dt.bfloat16
    f8 = mybir.dt.float8e4
    f32 = mybir.dt.float32

    nc = bacc.Bacc("TRN2", target_bir_lowering=False, debug=False, num_devices=8)

    xt = nc.dram_tensor("xt", [D, S], bf16, kind="ExternalInput").ap()
    bias = nc.dram_tensor("bias", [P, 4], f32, kind="ExternalInput").ap()
    masks = nc.dram_tensor("masks", [P, 4 * QT], bf16, kind="ExternalInput").ap()
    wvt = nc.dram_tensor("wvt", [D, GD], bf16, kind="ExternalInput").ap()
    wota0 = nc.dram_tensor("wota0", [P, D], bf16, kind="ExternalInput").ap()
    wota1 = nc.dram_tensor("wota1", [65, D], bf16, kind="ExternalInput").ap()
    zeros8 = nc.dram_tensor("zeros8", [P, S], f8, kind="ExternalInput").ap()
    vones = nc.dram_tensor("vones", [P, NKB, GH, 65], bf16,
                           kind="ExternalInput").ap()
    out = nc.dram_tensor("out", [SL, D], f32, kind="ExternalOutput").ap()
    if DRPROJ:
        xt8 = nc.dram_tensor("xt8", [D, S], f8, kind="ExternalInput").ap()
        xtq8 = nc.dram_tensor("xtq8", [D, SL], f8, kind="ExternalInput").ap()
        wq2 = nc.dram_tensor("wq2", [P, NC, 2, GD], f8, kind="ExternalInput").ap()
        wk2 = nc.dram_tensor("wk2", [P, NC, 2, GD], f8, kind="ExternalInput").ap()
    else:
        xtq = nc.dram_tensor("xtq", [D, SL], bf16, kind="ExternalInput").ap()
        wqt = nc.dram_tensor("wqt", [D, GD], bf16, kind="ExternalInput").ap()
        wkt = nc.dram_tensor("wkt", [D, GD], bf16, kind="ExternalInput").ap()

    Exp = mybir.ActivationFunctionType.Exp
    mult = mybir.AluOpType.mult
    add = mybir.AluOpType.add
    DR = mybir.MatmulPerfMode.DoubleRow

    with tile.TileContext(nc) as tc, ExitStack() as ctx:
        const = ctx.enter_context(tc.tile_pool(name="const", bufs=1))

        # ---- persistent SBUF tiles ----
        xt_sb = const.tile([P, NC, S], bf16, tag="xt")
        wvt_sb = const.tile([P, NC, GD], bf16, tag="wvt")
        wota0_sb = const.tile([P, D], bf16, tag="wota0")
        wota1_sb = const.tile([65, D], bf16, tag="wota1")
        bias_sb = const.tile([P, 4], f32, tag="bias")
        mask_sb = const.tile([P, 4, QT], bf16, tag="masks")
        if DRPROJ:
            xt8_sb = const.tile([P, NC, S], f8, tag="xt8")
            xtq8_sb = const.tile([P, NC, SL], f8, tag="xtq8")
            wq2_sb = const.tile([P, NC, 2, GD], f8, tag="wq2")
            wk2_sb = const.tile([P, NC, 2, GD], f8, tag="wk2")
        else:
            xtq_sb = const.tile([P, NC, SL], bf16, tag="xtq")
            wqt_sb = const.tile([P, NC, GD], bf16, tag="wqt")
            wkt_sb = const.tile([P, NC, GD], bf16, tag="wkt")
        # fp8 K cache, 3 heads packed into (partition-half, rail):
        #   (p 0:64,  rail 0) = head0 dims, (p 64:128, rail 0) = head1 dims,
        #   (p 0:64,  rail 1) = head2 dims, (p 64:128, rail 1) = ZERO.
        k8_sb = const.tile([P, 2, S], f8, tag="k8")
        # per-head fp8 Q, same packing; zeros everywhere except own region so
        # the shared K stationary operand only contracts the own head's dims.
        q8_sb = [const.tile([P, 2, SL], f8, tag=f"q8{h}", name=f"q8{h}")
                 for h in range(GH)]
        # V per key block: [128 k-part, kb, head, 65] with col 64 = 1.0
        v_sb = const.tile([P, NKB, GH, 65], bf16, tag="vall")

        # ---- weights on the gpsimd (SWDGE) queue, need-ordered ----
        # zero fills FIRST: the queue is FIFO, and everything the first QK
        # matmuls formally depend on (bias/weights -> psум -> evictions) sits
        # behind them, so the zero regions are guaranteed resident even if
        # the range tracker misses the partition-sliced overlap.
        for h in range(GH):
            nc.gpsimd.dma_start(q8_sb[h][:],
                                zeros8.rearrange("p (a b) -> p a b", a=2))
        nc.gpsimd.dma_start(k8_sb[:, 0, :], zeros8[:, :])
        nc.gpsimd.dma_start(k8_sb[:, 1, :], zeros8[:, :])
        nc.gpsimd.dma_start(bias_sb[:], bias[:])
        if DRPROJ:
            nc.gpsimd.dma_start(wq2_sb[:], wq2[:])
            nc.gpsimd.dma_start(wk2_sb[:], wk2[:])
        else:
            nc.gpsimd.dma_start(wqt_sb[:], wqt.rearrange("(c p) d -> p c d", p=P))
            nc.gpsimd.dma_start(wkt_sb[:], wkt.rearrange("(c p) d -> p c d", p=P))
        nc.gpsimd.dma_start(wvt_sb[:], wvt.rearrange("(c p) d -> p c d", p=P))
        nc.gpsimd.dma_start(mask_sb[:], masks.rearrange("p (w x) -> p w x", x=QT))
        nc.gpsimd.dma_start(wota0_sb[:], wota0[:])
        nc.gpsimd.dma_start(wota1_sb[:], wota1[:])

        # ---- inputs + region-exact zero fills (SP queue, JIT-ordered) ----
        # zero fills cover exactly the q8/k8 regions evictions never touch,
        # so they don't fence the eviction writes.
        xt_r = xt.rearrange("(c p) s -> p c s", p=P)
        if DRPROJ:
            xt8_r = xt8.rearrange("(c p) s -> p c s", p=P)
            xtq8_r = xtq8.rearrange("(c p) s -> p c s", p=P)
        else:
            xtq_r = xtq.rearrange("(c p) s -> p c s", p=P)

        def emit_windows(qg):
            qw = slice(qg * QGS, (qg + 1) * QGS)
            if DRPROJ:
                nc.sync.dma_start(xtq8_sb[:, :, qw], xtq8_r[:, :, qw])
            else:
                nc.sync.dma_start(xtq_sb[:, :, qw], xtq_r[:, :, qw])
            for kg in (2 * qg, 2 * qg + 1):
                kw = slice(kg * QGS, (kg + 1) * QGS)
                if DRPROJ:
                    nc.sync.dma_start(xt8_sb[:, :, kw], xt8_r[:, :, kw])
                nc.sync.dma_start(xt_sb[:, :, kw], xt_r[:, :, kw])

        nc.sync.dma_start(v_sb[:, 0:8], vones[:, 0:8])
        emit_windows(0)
        nc.sync.dma_start(v_sb[:, 8:NKB], vones[:, 8:NKB])
        for qg in range(1, QG):
            emit_windows(qg)

        evsc = 1.0 / WSCALE if DRPROJ else 1.0

        def proj_mms(ps, w2_sb, wt_sb, x8_sb, x_sb, cols, mrange):
            """contraction loop for one Q/K projection psum"""
            n = cols.stop - cols.start
            for c in range(NC):
                if DRPROJ:
                    rhs = (x8_sb[:, c, cols].unsqueeze(1)
                           .broadcast_to([P, 2, n]))
                    nc.tensor.matmul(ps[:], w2_sb[:, c, :, mrange], rhs,
                                     start=(c == 0), stop=(c == NC - 1),
                                     perf_mode=DR)
                else:
                    nc.tensor.matmul(ps[:], wt_sb[:, c, mrange],
                                     x_sb[:, c, cols],
                                     start=(c == 0), stop=(c == NC - 1))

        with tc.tile_pool(name="sc_ps", bufs=2, space="PSUM") as scps, \
             tc.tile_pool(name="pv_ps", bufs=3, space="PSUM") as pvps, \
             tc.tile_pool(name="pa_ps", bufs=1, space="PSUM") as paps, \
             tc.tile_pool(name="pt", bufs=8) as ptpool, \
             tc.tile_pool(name="stk", bufs=2) as stkpool, \
             tc.tile_pool(name="nrm", bufs=4) as nrmpool, \
             tc.tile_pool(name="oev", bufs=4) as oevpool:

            def qprojA(qg, use_sc):
                qw = slice(qg * QGS, (qg + 1) * QGS)
                if use_sc:
                    ps = scps.tile([P, 2 * QGS], f32, tag="sc",
                                   name=f"qps{qg}")[:, 0:QGS]
                else:
                    ps = paps.tile([P, QGS], f32, tag="pa", name=f"qps{qg}")[:]
                proj_mms(ps, DRPROJ and wq2_sb, not DRPROJ and wqt_sb,
                         DRPROJ and xtq8_sb, not DRPROJ and xtq_sb,
                         qw, slice(0, 128))
                nc.vector.tensor_scalar(
                    q8_sb[0][0:64, 0, qw], ps[0:64, :],
                    evsc, bias_sb[0:64, 0:1], mult, add)
                nc.vector.tensor_scalar(
                    q8_sb[1][64:128, 0, qw], ps[64:128, :],
                    evsc, bias_sb[64:128, 0:1], mult, add)

            def qprojB(qg):
                qw = slice(qg * QGS, (qg + 1) * QGS)
                ps2 = paps.tile([64, QGS], f32, tag="pa", name=f"qps2_{qg}")
                proj_mms(ps2[:], DRPROJ and wq2_sb, not DRPROJ and wqt_sb,
                         DRPROJ and xtq8_sb, not DRPROJ and xtq_sb,
                         qw, slice(128, 192))
                nc.vector.tensor_scalar(
                    q8_sb[2][0:64, 1, qw], ps2[:],
                    evsc, bias_sb[0:64, 1:2], mult, add)

            def kprojA(kg, use_sc):
                kcols = slice(kg * QGS, (kg + 1) * QGS)
                if use_sc:
                    ps = scps.tile([P, 2 * QGS], f32, tag="sc",
                                   name=f"kps{kg}")[:, 0:QGS]
                else:
                    ps = paps.tile([P, QGS], f32, tag="pa", name=f"kps{kg}")[:]
                proj_mms(ps, DRPROJ and wk2_sb, not DRPROJ and wkt_sb,
                         DRPROJ and xt8_sb, not DRPROJ and xt_sb,
                         kcols, slice(0, 128))
                nc.vector.tensor_scalar(
                    k8_sb[:, 0, kcols], ps[:],
                    evsc, bias_sb[:, 2:3], mult, add)

            def kprojB(kg):
                kcols = slice(kg * QGS, (kg + 1) * QGS)
                ps2 = paps.tile([64, QGS], f32, tag="pa", name=f"kps2_{kg}")
                proj_mms(ps2[:], DRPROJ and wk2_sb, not DRPROJ and wkt_sb,
                         DRPROJ and xt8_sb, not DRPROJ and xt_sb,
                         kcols, slice(128, 192))
                nc.vector.tensor_scalar(
                    k8_sb[0:64, 1, kcols], ps2[:],
                    evsc, bias_sb[0:64, 3:4], mult, add)

            def vproj(kb):
                psv = paps.tile([P, GD], f32, tag="pa", name=f"vps{kb}")
                for c in range(NC):
                    nc.tensor.matmul(
                        psv[:], xt_sb[:, c, kb * P:(kb + 1) * P],
                        wvt_sb[:, c, :],
                        start=(c == 0), stop=(c == NC - 1))
                nc.vector.tensor_copy(
                    v_sb[:, kb, :, 0:64],
                    psv[:].rearrange("p (h d) -> p h d", d=HD))

            # -- deadline-paced projection feed --------------------------
            # each entry: (t_deadline, m_deadline, emit_fn); the attention
            # m-loop emits every group whose deadline has arrived.
            feed = []
            for qg in range(QG):
                tq = max(0, 2 * qg - 1)
                if qg > 0:  # qg0's A-halves run at startup on the score banks
                    feed.append((tq, 0, lambda qg=qg: qprojA(qg, False)))
                feed.append((tq, 0, lambda qg=qg: qprojB(qg)))
                for kg in (2 * qg, 2 * qg + 1):
                    tk, mk = kg, max(0, 4 * kg - 3)
                    if kg > 0:
                        feed.append((tk, mk, lambda kg=kg: kprojA(kg, False)))
                    feed.append((tk, mk, lambda kg=kg: kprojB(kg)))
                for kb in range(8 * qg, 8 * qg + 8):
                    feed.append((kb // 4, max(0, kb - 2), lambda kb=kb: vproj(kb)))
            feed.sort(key=lambda e: (e[0], e[1]))

            def run_feed(t, m):
                while feed and (feed[0][0], feed[0][1]) <= (t, m):
                    feed.pop(0)[2]()

            # startup: Q0 / K-kg0-A first, rest from feed
            qprojA(0, False)
            kprojA(0, False)

            def attention(t):
                kcnt = 4 * (t + 1)
                pv = [pvps.tile([65, QGS], f32, tag="pv", name=f"pv{t}_{h}")
                      for h in range(GH)]
                for m in range(kcnt):
                    run_feed(t, m)
                    # causal query-suffix trim: for key block m, local query
                    # tiles j < ceil((m-1)/2) - 2t are entirely below the
                    # diagonal for BOTH parities; data masks handle leftovers.
                    q0 = P * (max(0, -(-(m - 1) // 2) - 2 * t) if m > 0 else 0)
                    sc = scps.tile([P, 2 * QGS], f32, tag="sc",
                                   name=f"sc{t}_{m}")
                    sc_v = sc[:].rearrange("p (h x) -> p h x", x=QT)
                    kblk = k8_sb[:, :, m * P:(m + 1) * P]
                    for h in range(GH):
                        nc.tensor.matmul(
                            sc_v[:, h, q0:], kblk,
                            q8_sb[h][:, :, t * QT + q0:(t + 1) * QT],
                            start=True, stop=True, perf_mode=DR)
                    pt = ptpool.tile([P, GH, QT], bf16, tag="pt")
                    # scores are raw q.k; the 1/sqrt(64) lives in the Exp scale
                    nc.scalar.activation(pt[:, :, q0:], sc_v[:, 0:GH, q0:],
                                         Exp, scale=0.125)
                    if m >= 4 * t:
                        # multiplicative 0/1 causal mask on the probabilities
                        w = m - 4 * t
                        span = P * (w // 2 + 1)
                        if span > q0:
                            for h in range(GH):
                                nc.gpsimd.tensor_tensor(
                                    pt[:, h, q0:span], pt[:, h, q0:span],
                                    mask_sb[:, w, q0:span], mult)
                    for h in range(GH):
                        nc.tensor.matmul(
                            pv[h][:, q0:QT], v_sb[:, m, h, :], pt[:, h, q0:],
                            start=(m == 0), stop=(m == kcnt - 1))
                # normalize: out_h = pv_h[0:64] / pv_h[64] ; stack for out-proj
                stk0 = stkpool.tile([P, QT], bf16, tag="sc0")
                stk1 = stkpool.tile([65, QT], bf16, tag="sc1")
                for h in range(GH):
                    recip = nrmpool.tile([1, QT], bf16, tag="recip")
                    with nc.allow_low_precision(
                            reason="bf16 softmax denominators, ~0.4% rel"):
                        nc.vector.reciprocal(recip[:], pv[h][64:65, 0:QT])
                    bcast = nrmpool.tile([65, QT], bf16, tag="bcast")
                    nc.gpsimd.partition_broadcast(bcast[:], recip[:])
                    if h < 2:
                        nc.vector.tensor_tensor(
                            stk0[h * 64:(h + 1) * 64, :], pv[h][0:64, 0:QT],
                            bcast[0:64, :], mult)
                    else:
                        # rows 0:64 = normalized h2; row 64 = denom/denom ~ 1
                        nc.vector.tensor_tensor(
                            stk1[:], pv[h][0:65, 0:QT], bcast[:], mult)
                # output projection for this group's 2 query tiles; the last
                # group spreads over the freed scores/pv banks and splits the
                # store so copy/DMA overlap.
                last = t == NT - 1
                for jj in range(2):
                    if last:
                        op0 = pvps.tile([P, 384], f32, tag="pv",
                                        name=f"op0_{t}_{jj}")
                        op1 = scps.tile([P, 2 * QGS], f32, tag="sc",
                                        name=f"op1_{t}_{jj}")[:, 0:384]
                    else:
                        op0 = paps.tile([P, 384], f32, tag="pa",
                                        name=f"op0_{t}_{jj}")[:]
                        op1 = paps.tile([P, 384], f32, tag="pa",
                                        name=f"op1_{t}_{jj}")[:]
                    for half, op in ((0, op0), (1, op1)):
                        nc.tensor.matmul(
                            op[:] if not last else op,
                            stk0[:, jj * P:(jj + 1) * P],
                            wota0_sb[:, half * 384:(half + 1) * 384],
                            start=True, stop=False)
                        nc.tensor.matmul(
                            op[:] if not last else op,
                            stk1[:, jj * P:(jj + 1) * P],
                            wota1_sb[:, half * 384:(half + 1) * 384],
                            start=False, stop=True)
                    jq = 2 * t + jj
                    oe = oevpool.tile([P, D], f32, tag="oe")
                    if last:
                        nc.vector.tensor_copy(oe[:, 0:384], op0)
                        nc.sync.dma_start(out[jq * P:(jq + 1) * P, 0:384],
                                          oe[:, 0:384])
                        nc.vector.tensor_copy(oe[:, 384:768], op1)
                        nc.sync.dma_start(out[jq * P:(jq + 1) * P, 384:768],
                                          oe[:, 384:768])
                    else:
                        nc.vector.tensor_copy(oe[:, 0:384], op0)
                        nc.vector.tensor_copy(oe[:, 384:768], op1)
                        nc.sync.dma_start(out[jq * P:(jq + 1) * P, :], oe[:])

            for t in range(NT):
                attention(t)
            # flush any remaining feed entries (shouldn't be any)
            while feed:
                feed.pop(0)[2]()

    nc.compile()
    return nc


def _host_prep(inputs, Wq, bq, Wk, bk, Wv, bv, Wo, bo):
    import ml_dtypes

    bf16 = ml_dtypes.bfloat16
    f8 = ml_dtypes.float8_e4m3
    X = np.asarray(inputs, np.float32).reshape(S, D)
    XT = np.ascontiguousarray(X.T)                      # [768, 4096]
    XT_bf = XT.astype(bf16)
    XT_f8 = XT.astype(f8)
    # query-set gathers: blocks s, s+2, ... of 32 128-col blocks
    XTb = XT.reshape(D, NKB // 2, 2, P)
    XTq = [np.ascontiguousarray(XTb[:, :, s, :].reshape(D, SL)) for s in range(2)]

    # per-core multiplicative causal masks [128, 4, 256], 1=keep 0=drop
    # (S_T layout: k on partitions, q on free dim); w = m - 4t
    tri = (np.arange(P)[None, :] >= np.arange(P)[:, None]).astype(np.float32)
    mk = []
    for s_ in range(2):
        m = np.ones((P, 4, QT), np.float32)
        for w in range(4):
            npref = max(0, -(-(w - s_) // 2))  # ceil((w - s)/2) clamped at 0
            m[:, w, :P * npref] = 0.0
            if w >= s_ and (w - s_) % 2 == 0:
                dblk = (w - s_) // 2
                m[:, w, dblk * P:(dblk + 1) * P] = tri
        mk.append(np.ascontiguousarray(m.reshape(P, 4 * QT)).astype(bf16))

    zeros8 = np.zeros((P, S), f8)
    vones_h = np.ones((P, NKB, GH, 65), bf16)

    def dual_rail(WT):  # [768, 192] f32 -> [128, 6, 2, 192] fp8 (hi, lo)
        Ws = WT * WSCALE
        hi = Ws.astype(f8)
        lo = (Ws - hi.astype(np.float32)).astype(f8)
        w2 = np.zeros((P, NC, 2, GD), np.float32)
        for c in range(NC):
            w2[:, c, 0, :] = hi[c * P:(c + 1) * P].astype(np.float32)
            w2[:, c, 1, :] = lo[c * P:(c + 1) * P].astype(np.float32)
        return np.ascontiguousarray(w2).astype(f8)

    in_maps = []
    for g in range(NG):
        hs = slice(GD * g, GD * (g + 1))
        WqT = np.ascontiguousarray(Wq[hs, :].T).astype(np.float32)
        WkT = np.ascontiguousarray(Wk[hs, :].T).astype(np.float32)
        WvT = np.ascontiguousarray(Wv[hs, :].T).astype(bf16)
        WoT = np.ascontiguousarray(Wo[:, hs].T).astype(np.float32)  # [192, 768]
        bo_g = bv[hs].astype(np.float32) @ WoT
        if g == 0:
            bo_g = bo_g + bo.astype(np.float32)
        wota = np.concatenate([WoT, bo_g[None, :]], axis=0)  # [193, 768]
        wota0 = np.ascontiguousarray(wota[0:P]).astype(bf16)
        wota1 = np.ascontiguousarray(wota[P:]).astype(bf16)
        bias_t = np.zeros((P, 4), np.float32)
        bias_t[:, 0] = bq[hs][0:128]
        bias_t[0:64, 1] = bq[hs][128:192]
        bias_t[:, 2] = bk[hs][0:128]
        bias_t[0:64, 3] = bk[hs][128:192]
        base = {
            "xt": XT_bf, "wvt": WvT,
            "wota0": wota0, "wota1": wota1,
            "bias": bias_t, "zeros8": zeros8, "vones": vones_h,
        }
        if DRPROJ:
            base["xt8"] = XT_f8
            base["wq2"] = dual_rail(WqT)
            base["wk2"] = dual_rail(WkT)
        else:
            base["wqt"] = WqT.astype(bf16)
            base["wkt"] = WkT.astype(bf16)
        for s_ in range(2):
            m = dict(base)
            m["masks"] = mk[s_]
            if DRPROJ:
                m["xtq8"] = XTq[s_].astype(f8)
            else:
                m["xtq"] = XTq[s_].astype(bf16)
            in_maps.append(m)
    return in_maps


def _gather(results):
    out = np.zeros((S, D), np.float32)
    ov = out.reshape(NQB, 2, P, D)
    for s_ in range(2):
        acc = np.zeros((SL, D), np.float32)
        for g in range(NG):
            acc += np.asarray(results[2 * g + s_]["out"], np.float32)
        ov[:, s_, :, :] = acc.reshape(NQB, P, D)
    return out.reshape(1, S, D)


def kernel(inputs, Wq, bq, Wk, bk, Wv, bv, Wo, bo):
    from concourse.bass_utils import run_bass_kernel_spmd

    if "nc" not in _CACHE:
        _CACHE["nc"] = _build_program()
    nc = _CACHE["nc"]
    in_maps = _host_prep(
        np.asarray(inputs), np.asarray(Wq), np.asarray(bq), np.asarray(Wk),
        np.asarray(bk), np.asarray(Wv), np.asarray(bv), np.asarray(Wo),
        np.asarray(bo))
    # core order: core = 2*g + s; in_maps was built g-major with s inner.
    res = run_bass_kernel_spmd(nc, in_maps, list(range(8))).results
    return _gather(res)


# revision 15
# speedup vs baseline: 1.1070x; 1.1070x over previous
"""Multi-head causal self-attention (D=768, H=12, S=4096) on 8 Trainium2 cores.

Sharding: 4 head-groups (3 heads each) x 2 interleaved query-sets.
Core c = 2*g + s owns head-group g (heads 3g..3g+2) and query 128-row
blocks s, s+2, s+4, ... (even/odd interleave balances the causal
triangle).  Every core runs the SAME program; per-core behaviour is
driven entirely by input data.  Each core produces a partial [2048, 768]
output (its heads pushed through its slice of Wo, all biases folded
in); the host sums the 4 group partials per query-set and re-interleaves
rows.

v3.1 key points:
  - QK^T matmuls are fp8e4 DoubleRow (half the PE columns): Q and K live
    in zero-padded [128, 2, *] fp8 tiles so all three heads share one
    stationary operand; zero regions kill cross-head terms.  The zero
    fills are region-exact DMAs so they never fence the evictions.
  - Q/K projections are fp8 DoubleRow with dual-rail (hi+lo) fp8 weights.
  - attention runs on 256-query tiles with all 3 heads' scores packed in
    ONE psum tile (2 banks, double-buffered bank-disjoint; PE-write +
    ACT-read on one bank is fatal), one exp instr per key block; psum =
    4 (scores) + 3 (pv) + 1 (projection) banks.
  - projection work is sliced into accumulation groups and fed into the
    attention m-loop just-in-time (deadline-paced), so the PE stream
    interleaves projections under the ACT-bound attention instead of
    bursting between groups.
  - normalize uses gpsimd partition_broadcast + a fused psum*bf16 DVE
    multiply; causal-mask multiplies run on GPSIMD; V/P/O stay bf16
    (fp8 there costs ~2.4% output noise).
"""

import numpy as np

D = 768
S = 4096
H = 12
HD = 64
NG = 4          # head groups
GH = 3          # heads per group
GD = GH * HD    # 192 dims per group
SL = S // 2     # local queries per core (2048)
P = 128
NC = D // P     # 6 contraction chunks
QG = 4          # projection chunks (512 q each)
QGS = 512
NT = 8          # attention query groups (256 q each)
QT = 256
NKB = S // P    # 32 key blocks
NQB = SL // P   # 16 local query tiles

DRPROJ = True   # Q/K projections via fp8 dual-rail DoubleRow
WSCALE = 16.0   # pow-2 scale keeping dual-rail fp8 weights in normal range

_CACHE = {}


def _build_program():
    import concourse.bacc as bacc
    import concourse.mybir as mybir
    import concourse.tile as tile
    from contextlib import ExitStack

    bf16 = mybir.dt.bfloat16
    f8 = mybir.dt.float8e4
    f32 = mybir.dt.float32

    nc = bacc.Bacc("TRN2", target_bir_lowering=False, debug=False, num_devices=8)

    xt = nc.dram_tensor("xt", [D, S], bf16, kind="ExternalInput").ap()
    bias = nc.dram_tensor("bias", [P, 4], f32, kind="ExternalInput").ap()
    masks = nc.dram_tensor("masks", [P, 4 * QT], bf16, kind="ExternalInput").ap()
    wvt = nc.dram_tensor("wvt", [D, GD], bf16, kind="ExternalInput").ap()
    wota0 = nc.dram_tensor("wota0", [P, D], bf16, kind="ExternalInput").ap()
    wota1 = nc.dram_tensor("wota1", [65, D], bf16, kind="ExternalInput").ap()
    zeros8 = nc.dram_tensor("zeros8", [P, S], f8, kind="ExternalInput").ap()
    vones = nc.dram_tensor("vones", [P, NKB, GH, 65], bf16,
                           kind="ExternalInput").ap()
    out = nc.dram_tensor("out", [SL, D], f32, kind="ExternalOutput").ap()
    if DRPROJ:
        xt8 = nc.dram_tensor("xt8", [D, S], f8, kind="ExternalInput").ap()
        xtq8 = nc.dram_tensor("xtq8", [D, SL], f8, kind="ExternalInput").ap()
        wq2 = nc.dram_tensor("wq2", [P, NC, 2, GD], f8, kind="ExternalInput").ap()
        wk2 = nc.dram_tensor("wk2", [P, NC, 2, GD], f8, kind="ExternalInput").ap()
    else:
        xtq = nc.dram_tensor("xtq", [D, SL], bf16, kind="ExternalInput").ap()
        wqt = nc.dram_tensor("wqt", [D, GD], bf16, kind="ExternalInput").ap()
        wkt = nc.dram_tensor("wkt", [D, GD], bf16, kind="ExternalInput").ap()

    Exp = mybir.ActivationFunctionType.Exp
    mult = mybir.AluOpType.mult
    add = mybir.AluOpType.add
    DR = mybir.MatmulPerfMode.DoubleRow

    with tile.TileContext(nc) as tc, ExitStack() as ctx:
        const = ctx.enter_context(tc.tile_pool(name="const", bufs=1))

        # ---- persistent SBUF tiles ----
        xt_sb = const.tile([P, NC, S], bf16, tag="xt")
        wvt_sb = const.tile([P, NC, GD], bf16, tag="wvt")
        wota0_sb = const.tile([P, D], bf16, tag="wota0")
        wota1_sb = const.tile([65, D], bf16, tag="wota1")
        bias_sb = const.tile([P, 4], f32, tag="bias")
        mask_sb = const.tile([P, 4, QT], bf16, tag="masks")
        if DRPROJ:
            xt8_sb = const.tile([P, NC, S], f8, tag="xt8")
            xtq8_sb = const.tile([P, NC, SL], f8, tag="xtq8")
            wq2_sb = const.tile([P, NC, 2, GD], f8, tag="wq2")
            wk2_sb = const.tile([P, NC, 2, GD], f8, tag="wk2")
        else:
            xtq_sb = const.tile([P, NC, SL], bf16, tag="xtq")
            wqt_sb = const.tile([P, NC, GD], bf16, tag="wqt")
            wkt_sb = const.tile([P, NC, GD], bf16, tag="wkt")
        # fp8 K cache, 3 heads packed into (partition-half, rail):
        #   (p 0:64,  rail 0) = head0 dims, (p 64:128, rail 0) = head1 dims,
        #   (p 0:64,  rail 1) = head2 dims, (p 64:128, rail 1) = ZERO.
        k8_sb = const.tile([P, 2, S], f8, tag="k8")
        # per-head fp8 Q, same packing; zeros everywhere except own region so
        # the shared K stationary operand only contracts the own head's dims.
        q8_sb = [const.tile([P, 2, SL], f8, tag=f"q8{h}", name=f"q8{h}")
                 for h in range(GH)]
        # V per key block: [128 k-part, kb, head, 65] with col 64 = 1.0
        v_sb = const.tile([P, NKB, GH, 65], bf16, tag="vall")

        # ---- weights on the gpsimd (SWDGE) queue, need-ordered ----
        nc.gpsimd.dma_start(bias_sb[:], bias[:])
        if DRPROJ:
            nc.gpsimd.dma_start(wq2_sb[:], wq2[:])
        else:
            nc.gpsimd.dma_start(wqt_sb[:], wqt.rearrange("(c p) d -> p c d", p=P))
        nc.gpsimd.dma_start(mask_sb[:], masks.rearrange("p (w x) -> p w x", x=QT))
        if DRPROJ:
            nc.gpsimd.dma_start(wk2_sb[:], wk2[:])
        else:
            nc.gpsimd.dma_start(wkt_sb[:], wkt.rearrange("(c p) d -> p c d", p=P))
        nc.gpsimd.dma_start(wvt_sb[:], wvt.rearrange("(c p) d -> p c d", p=P))
        nc.gpsimd.dma_start(wota0_sb[:], wota0[:])
        nc.gpsimd.dma_start(wota1_sb[:], wota1[:])

        # ---- inputs + region-exact zero fills (SP queue, JIT-ordered) ----
        # zero fills cover exactly the q8/k8 regions evictions never touch,
        # so they don't fence the eviction writes.
        xt_r = xt.rearrange("(c p) s -> p c s", p=P)
        if DRPROJ:
            xt8_r = xt8.rearrange("(c p) s -> p c s", p=P)
            xtq8_r = xtq8.rearrange("(c p) s -> p c s", p=P)
        else:
            xtq_r = xtq.rearrange("(c p) s -> p c s", p=P)

        def emit_windows(qg):
            qw = slice(qg * QGS, (qg + 1) * QGS)
            if DRPROJ:
                nc.sync.dma_start(xtq8_sb[:, :, qw], xtq8_r[:, :, qw])
            else:
                nc.sync.dma_start(xtq_sb[:, :, qw], xtq_r[:, :, qw])
            for kg in (2 * qg, 2 * qg + 1):
                kw = slice(kg * QGS, (kg + 1) * QGS)
                if DRPROJ:
                    nc.sync.dma_start(xt8_sb[:, :, kw], xt8_r[:, :, kw])
                nc.sync.dma_start(xt_sb[:, :, kw], xt_r[:, :, kw])

        qw0 = slice(0, QGS)
        if DRPROJ:
            nc.sync.dma_start(xtq8_sb[:, :, qw0], xtq8_r[:, :, qw0])
        else:
            nc.sync.dma_start(xtq_sb[:, :, qw0], xtq_r[:, :, qw0])
        for h in range(GH):
            nc.sync.dma_start(q8_sb[h][:],
                              zeros8.rearrange("p (a b) -> p a b", a=2))
        if DRPROJ:
            nc.sync.dma_start(xt8_sb[:, :, qw0], xt8_r[:, :, qw0])
        nc.sync.dma_start(k8_sb[:, 0, :], zeros8[:, :])
        nc.sync.dma_start(k8_sb[:, 1, :], zeros8[:, :])
        nc.sync.dma_start(xt_sb[:, :, qw0], xt_r[:, :, qw0])
        nc.sync.dma_start(v_sb[:, 0:8], vones[:, 0:8])
        kw1 = slice(QGS, 2 * QGS)
        if DRPROJ:
            nc.sync.dma_start(xt8_sb[:, :, kw1], xt8_r[:, :, kw1])
        nc.sync.dma_start(xt_sb[:, :, kw1], xt_r[:, :, kw1])
        nc.sync.dma_start(v_sb[:, 8:NKB], vones[:, 8:NKB])
        for qg in range(1, QG):
            emit_windows(qg)

        evsc = 1.0 / WSCALE if DRPROJ else 1.0

        def proj_mms(ps, w2_sb, wt_sb, x8_sb, x_sb, cols, mrange):
            """contraction loop for one Q/K projection psum"""
            n = cols.stop - cols.start
            for c in range(NC):
                if DRPROJ:
                    rhs = (x8_sb[:, c, cols].unsqueeze(1)
                           .broadcast_to([P, 2, n]))
                    nc.tensor.matmul(ps[:], w2_sb[:, c, :, mrange], rhs,
                                     start=(c == 0), stop=(c == NC - 1),
                                     perf_mode=DR)
                else:
                    nc.tensor.matmul(ps[:], wt_sb[:, c, mrange],
                                     x_sb[:, c, cols],
                                     start=(c == 0), stop=(c == NC - 1))

        with tc.tile_pool(name="sc_ps", bufs=2, space="PSUM") as scps, \
             tc.tile_pool(name="pv_ps", bufs=3, space="PSUM") as pvps, \
             tc.tile_pool(name="pa_ps", bufs=1, space="PSUM") as paps, \
             tc.tile_pool(name="pt", bufs=8) as ptpool, \
             tc.tile_pool(name="stk", bufs=2) as stkpool, \
             tc.tile_pool(name="nrm", bufs=4) as nrmpool, \
             tc.tile_pool(name="oev", bufs=4) as oevpool:

            def qprojA(qg, use_sc):
                qw = slice(qg * QGS, (qg + 1) * QGS)
                if use_sc:
                    ps = scps.tile([P, 2 * QGS], f32, tag="sc",
                                   name=f"qps{qg}")[:, 0:QGS]
                else:
                    ps = paps.tile([P, QGS], f32, tag="pa", name=f"qps{qg}")[:]
                proj_mms(ps, DRPROJ and wq2_sb, not DRPROJ and wqt_sb,
                         DRPROJ and xtq8_sb, not DRPROJ and xtq_sb,
                         qw, slice(0, 128))
                nc.vector.tensor_scalar(
                    q8_sb[0][0:64, 0, qw], ps[0:64, :],
                    evsc, bias_sb[0:64, 0:1], mult, add)
                nc.vector.tensor_scalar(
                    q8_sb[1][64:128, 0, qw], ps[64:128, :],
                    evsc, bias_sb[64:128, 0:1], mult, add)

            def qprojB(qg):
                qw = slice(qg * QGS, (qg + 1) * QGS)
                ps2 = paps.tile([64, QGS], f32, tag="pa", name=f"qps2_{qg}")
                proj_mms(ps2[:], DRPROJ and wq2_sb, not DRPROJ and wqt_sb,
                         DRPROJ and xtq8_sb, not DRPROJ and xtq_sb,
                         qw, slice(128, 192))
                nc.vector.tensor_scalar(
                    q8_sb[2][0:64, 1, qw], ps2[:],
                    evsc, bias_sb[0:64, 1:2], mult, add)

            def kprojA(kg, use_sc):
                kcols = slice(kg * QGS, (kg + 1) * QGS)
                if use_sc:
                    ps = scps.tile([P, 2 * QGS], f32, tag="sc",
                                   name=f"kps{kg}")[:, 0:QGS]
                else:
                    ps = paps.tile([P, QGS], f32, tag="pa", name=f"kps{kg}")[:]
                proj_mms(ps, DRPROJ and wk2_sb, not DRPROJ and wkt_sb,
                         DRPROJ and xt8_sb, not DRPROJ and xt_sb,
                         kcols, slice(0, 128))
                nc.vector.tensor_scalar(
                    k8_sb[:, 0, kcols], ps[:],
                    evsc, bias_sb[:, 2:3], mult, add)

            def kprojB(kg):
                kcols = slice(kg * QGS, (kg + 1) * QGS)
                ps2 = paps.tile([64, QGS], f32, tag="pa", name=f"kps2_{kg}")
                proj_mms(ps2[:], DRPROJ and wk2_sb, not DRPROJ and wkt_sb,
                         DRPROJ and xt8_sb, not DRPROJ and xt_sb,
                         kcols, slice(128, 192))
                nc.vector.tensor_scalar(
                    k8_sb[0:64, 1, kcols], ps2[:],
                    evsc, bias_sb[0:64, 3:4], mult, add)

            def vproj(kb):
                psv = paps.tile([P, GD], f32, tag="pa", name=f"vps{kb}")
                for c in range(NC):
                    nc.tensor.matmul(
                        psv[:], xt_sb[:, c, kb * P:(kb + 1) * P],
                        wvt_sb[:, c, :],
                        start=(c == 0), stop=(c == NC - 1))
                nc.vector.tensor_copy(
                    v_sb[:, kb, :, 0:64],
                    psv[:].rearrange("p (h d) -> p h d", d=HD))

            # -- deadline-paced projection feed --------------------------
            # each entry: (t_deadline, m_deadline, emit_fn); the attention
            # m-loop emits every group whose deadline has arrived.
            feed = []
            for qg in range(QG):
                tq = max(0, 2 * qg - 1)
                if qg > 0:  # qg0's A-halves run at startup on the score banks
                    feed.append((tq, 0, lambda qg=qg: qprojA(qg, False)))
                feed.append((tq, 0, lambda qg=qg: qprojB(qg)))
                for kg in (2 * qg, 2 * qg + 1):
                    tk, mk = kg, max(0, 4 * kg - 3)
                    if kg > 0:
                        feed.append((tk, mk, lambda kg=kg: kprojA(kg, False)))
                    feed.append((tk, mk, lambda kg=kg: kprojB(kg)))
                for kb in range(8 * qg, 8 * qg + 8):
                    feed.append((kb // 4, max(0, kb - 2), lambda kb=kb: vproj(kb)))
            feed.sort(key=lambda e: (e[0], e[1]))

            def run_feed(t, m):
                while feed and (feed[0][0], feed[0][1]) <= (t, m):
                    feed.pop(0)[2]()

            # startup: Q0 / K-kg0-A first, rest from feed
            qprojA(0, False)
            kprojA(0, False)

            def attention(t):
                kcnt = 4 * (t + 1)
                pv = [pvps.tile([65, QGS], f32, tag="pv", name=f"pv{t}_{h}")
                      for h in range(GH)]
                for m in range(kcnt):
                    run_feed(t, m)
                    # causal query-suffix trim: for key block m, local query
                    # tiles j < ceil((m-1)/2) - 2t are entirely below the
                    # diagonal for BOTH parities; data masks handle leftovers.
                    q0 = P * (max(0, -(-(m - 1) // 2) - 2 * t) if m > 0 else 0)
                    sc = scps.tile([P, 2 * QGS], f32, tag="sc",
                                   name=f"sc{t}_{m}")
                    sc_v = sc[:].rearrange("p (h x) -> p h x", x=QT)
                    kblk = k8_sb[:, :, m * P:(m + 1) * P]
                    for h in range(GH):
                        nc.tensor.matmul(
                            sc_v[:, h, q0:], kblk,
                            q8_sb[h][:, :, t * QT + q0:(t + 1) * QT],
                            start=True, stop=True, perf_mode=DR)
                    pt = ptpool.tile([P, GH, QT], bf16, tag="pt")
                    # scores are raw q.k; the 1/sqrt(64) lives in the Exp scale
                    nc.scalar.activation(pt[:, :, q0:], sc_v[:, 0:GH, q0:],
                                         Exp, scale=0.125)
                    if m >= 4 * t:
                        # multiplicative 0/1 causal mask on the probabilities
                        w = m - 4 * t
                        span = P * (w // 2 + 1)
                        if span > q0:
                            for h in range(GH):
                                nc.gpsimd.tensor_tensor(
                                    pt[:, h, q0:span], pt[:, h, q0:span],
                                    mask_sb[:, w, q0:span], mult)
                    for h in range(GH):
                        nc.tensor.matmul(
                            pv[h][:, q0:QT], v_sb[:, m, h, :], pt[:, h, q0:],
                            start=(m == 0), stop=(m == kcnt - 1))
                # normalize: out_h = pv_h[0:64] / pv_h[64] ; stack for out-proj
                stk0 = stkpool.tile([P, QT], bf16, tag="sc0")
                stk1 = stkpool.tile([65, QT], bf16, tag="sc1")
                for h in range(GH):
                    recip = nrmpool.tile([1, QT], bf16, tag="recip")
                    with nc.allow_low_precision(
                            reason="bf16 softmax denominators, ~0.4% rel"):
                        nc.vector.reciprocal(recip[:], pv[h][64:65, 0:QT])
                    bcast = nrmpool.tile([65, QT], bf16, tag="bcast")
                    nc.gpsimd.partition_broadcast(bcast[:], recip[:])
                    if h < 2:
                        nc.vector.tensor_tensor(
                            stk0[h * 64:(h + 1) * 64, :], pv[h][0:64, 0:QT],
                            bcast[0:64, :], mult)
                    else:
                        # rows 0:64 = normalized h2; row 64 = denom/denom ~ 1
                        nc.vector.tensor_tensor(
                            stk1[:], pv[h][0:65, 0:QT], bcast[:], mult)
                # output projection for this group's 2 query tiles; the last
                # group spreads over the freed scores/pv banks and splits the
                # store so copy/DMA overlap.
                last = t == NT - 1
                for jj in range(2):
                    op0 = pvps.tile([P, 384], f32, tag="pv",
                                    name=f"op0_{t}_{jj}")[:]
                    if last:
                        op1 = scps.tile([P, 2 * QGS], f32, tag="sc",
                                        name=f"op1_{t}_{jj}")[:, 0:384]
                    else:
                        op1 = paps.tile([P, 384], f32, tag="pa",
                                        name=f"op1_{t}_{jj}")[:]
                    for half, op in ((0, op0), (1, op1)):
                        nc.tensor.matmul(
                            op, stk0[:, jj * P:(jj + 1) * P],
                            wota0_sb[:, half * 384:(half + 1) * 384],
                            start=True, stop=False)
                        nc.tensor.matmul(
                            op, stk1[:, jj * P:(jj + 1) * P],
                            wota1_sb[:, half * 384:(half + 1) * 384],
                            start=False, stop=True)
                    jq = 2 * t + jj
                    oe = oevpool.tile([P, D], f32, tag="oe")
                    if last:
                        nc.vector.tensor_copy(oe[:, 0:384], op0)
                        nc.sync.dma_start(out[jq * P:(jq + 1) * P, 0:384],
                                          oe[:, 0:384])
                        nc.vector.tensor_copy(oe[:, 384:768], op1)
                        nc.sync.dma_start(out[jq * P:(jq + 1) * P, 384:768],
                                          oe[:, 384:768])
                    else:
                        nc.vector.tensor_copy(oe[:, 0:384], op0)
                        nc.vector.tensor_copy(oe[:, 384:768], op1)
                        nc.sync.dma_start(out[jq * P:(jq + 1) * P, :], oe[:])

            for t in range(NT):
                attention(t)
            # flush any remaining feed entries (shouldn't be any)
            while feed:
                feed.pop(0)[2]()

    nc.compile()
    return nc


def _host_prep(inputs, Wq, bq, Wk, bk, Wv, bv, Wo, bo):
    import ml_dtypes

    bf16 = ml_dtypes.bfloat16
    f8 = ml_dtypes.float8_e4m3
    X = np.asarray(inputs, np.float32).reshape(S, D)
    XT = np.ascontiguousarray(X.T)                      # [768, 4096]
    XT_bf = XT.astype(bf16)
    XT_f8 = XT.astype(f8)
    # query-set gathers: blocks s, s+2, ... of 32 128-col blocks
    XTb = XT.reshape(D, NKB // 2, 2, P)
    XTq = [np.ascontiguousarray(XTb[:, :, s, :].reshape(D, SL)) for s in range(2)]

    # per-core multiplicative causal masks [128, 4, 256], 1=keep 0=drop
    # (S_T layout: k on partitions, q on free dim); w = m - 4t
    tri = (np.arange(P)[None, :] >= np.arange(P)[:, None]).astype(np.float32)
    mk = []
    for s_ in range(2):
        m = np.ones((P, 4, QT), np.float32)
        for w in range(4):
            npref = max(0, -(-(w - s_) // 2))  # ceil((w - s)/2) clamped at 0
            m[:, w, :P * npref] = 0.0
            if w >= s_ and (w - s_) % 2 == 0:
                dblk = (w - s_) // 2
                m[:, w, dblk * P:(dblk + 1) * P] = tri
        mk.append(np.ascontiguousarray(m.reshape(P, 4 * QT)).astype(bf16))

    zeros8 = np.zeros((P, S), f8)
    vones_h = np.ones((P, NKB, GH, 65), bf16)

    def dual_rail(WT):  # [768, 192] f32 -> [128, 6, 2, 192] fp8 (hi, lo)
        Ws = WT * WSCALE
        hi = Ws.astype(f8)
        lo = (Ws - hi.astype(np.float32)).astype(f8)
        w2 = np.zeros((P, NC, 2, GD), np.float32)
        for c in range(NC):
            w2[:, c, 0, :] = hi[c * P:(c + 1) * P].astype(np.float32)
            w2[:, c, 1, :] = lo[c * P:(c + 1) * P].astype(np.float32)
        return np.ascontiguousarray(w2).astype(f8)

    in_maps = []
    for g in range(NG):
        hs = slice(GD * g, GD * (g + 1))
        WqT = np.ascontiguousarray(Wq[hs, :].T).astype(np.float32)
        WkT = np.ascontiguousarray(Wk[hs, :].T).astype(np.float32)
        WvT = np.ascontiguousarray(Wv[hs, :].T).astype(bf16)
        WoT = np.ascontiguousarray(Wo[:, hs].T).astype(np.float32)  # [192, 768]
        bo_g = bv[hs].astype(np.float32) @ WoT
        if g == 0:
            bo_g = bo_g + bo.astype(np.float32)
        wota = np.concatenate([WoT, bo_g[None, :]], axis=0)  # [193, 768]
        wota0 = np.ascontiguousarray(wota[0:P]).astype(bf16)
        wota1 = np.ascontiguousarray(wota[P:]).astype(bf16)
        bias_t = np.zeros((P, 4), np.float32)
        bias_t[:, 0] = bq[hs][0:128]
        bias_t[0:64, 1] = bq[hs][128:192]
        bias_t[:, 2] = bk[hs][0:128]
        bias_t[0:64, 3] = bk[hs][128:192]
        base = {
            "xt": XT_bf, "wvt": WvT,
            "wota0": wota0, "wota1": wota1,
            "bias": bias_t, "zeros8": zeros8, "vones": vones_h,
        }
        if DRPROJ:
            base["xt8"] = XT_f8
            base["wq2"] = dual_rail(WqT)
            base["wk2"] = dual_rail(WkT)
        else:
            base["wqt"] = WqT.astype(bf16)
            base["wkt"] = WkT.astype(bf16)
        for s_ in range(2):
            m = dict(base)
            m["masks"] = mk[s_]
            if DRPROJ:
                m["xtq8"] = XTq[s_].astype(f8)
            else:
                m["xtq"] = XTq[s_].astype(bf16)
            in_maps.append(m)
    return in_maps


def _gather(results):
    out = np.zeros((S, D), np.float32)
    ov = out.reshape(NQB, 2, P, D)
    for s_ in range(2):
        acc = np.zeros((SL, D), np.float32)
        for g in range(NG):
            acc += np.asarray(results[2 * g + s_]["out"], np.float32)
        ov[:, s_, :, :] = acc.reshape(NQB, P, D)
    return out.reshape(1, S, D)


def kernel(inputs, Wq, bq, Wk, bk, Wv, bv, Wo, bo):
    from concourse.bass_utils import run_bass_kernel_spmd

    if "nc" not in _CACHE:
        _CACHE["nc"] = _build_program()
    nc = _CACHE["nc"]
    in_maps = _host_prep(
        np.asarray(inputs), np.asarray(Wq), np.asarray(bq), np.asarray(Wk),
        np.asarray(bk), np.asarray(Wv), np.asarray(bv), np.asarray(Wo),
        np.asarray(bo))
    # core order: core = 2*g + s; in_maps was built g-major with s inner.
    res = run_bass_kernel_spmd(nc, in_maps, list(range(8))).results
    return _gather(res)


# revision 26
# speedup vs baseline: 1.1286x; 1.0195x over previous
"""Multi-head causal self-attention (D=768, H=12, S=4096) on 8 Trainium2 cores.

Sharding: 4 head-groups (3 heads each) x 2 interleaved query-sets.
Core c = 2*g + s owns head-group g (heads 3g..3g+2) and query 128-row
blocks s, s+2, s+4, ... (even/odd interleave balances the causal
triangle).  Every core runs the SAME program; per-core behaviour is
driven entirely by input data.  Each core produces a partial [2048, 768]
output (its heads pushed through its slice of Wo, all biases folded
in); the host sums the 4 group partials per query-set and re-interleaves
rows.

v3.1 key points:
  - QK^T matmuls are fp8e4 DoubleRow (half the PE columns): Q and K live
    in zero-padded [128, 2, *] fp8 tiles so all three heads share one
    stationary operand; zero regions kill cross-head terms.  The zero
    fills are region-exact DMAs so they never fence the evictions.
  - Q/K projections are fp8 DoubleRow with dual-rail (hi+lo) fp8 weights.
  - attention runs on 256-query tiles with all 3 heads' scores packed in
    ONE psum tile (2 banks, double-buffered bank-disjoint; PE-write +
    ACT-read on one bank is fatal), one exp instr per key block; psum =
    4 (scores) + 3 (pv) + 1 (projection) banks.
  - projection work is sliced into accumulation groups and fed into the
    attention m-loop just-in-time (deadline-paced), so the PE stream
    interleaves projections under the ACT-bound attention instead of
    bursting between groups.
  - normalize uses gpsimd partition_broadcast + a fused psum*bf16 DVE
    multiply; causal-mask multiplies run on GPSIMD; V/P/O stay bf16
    (fp8 there costs ~2.4% output noise).
"""

import numpy as np

D = 768
S = 4096
H = 12
HD = 64
NG = 4          # head groups
GH = 3          # heads per group
GD = GH * HD    # 192 dims per group
SL = S // 2     # local queries per core (2048)
P = 128
NC = D // P     # 6 contraction chunks
QG = 4          # projection chunks (512 q each)
QGS = 512
NT = 8          # attention query groups (256 q each)
QT = 256
NKB = S // P    # 32 key blocks
NQB = SL // P   # 16 local query tiles

DRPROJ = True   # Q/K projections via fp8 dual-rail DoubleRow
WSCALE = 16.0   # pow-2 scale keeping dual-rail fp8 weights in normal range

_CACHE = {}


def _build_program():
    import concourse.bacc as bacc
    import concourse.mybir as mybir
    import concourse.tile as tile
    from contextlib import ExitStack

    bf16 = mybir.dt.bfloat16
    f8 = mybir.dt.float8e4
    f32 = mybir.dt.float32

    nc = bacc.Bacc("TRN2", target_bir_lowering=False, debug=False, num_devices=8)

    xt = nc.dram_tensor("xt", [D, S], bf16, kind="ExternalInput").ap()
    bias = nc.dram_tensor("bias", [P, 4], f32, kind="ExternalInput").ap()
    masks = nc.dram_tensor("masks", [P, 4 * QT], bf16, kind="ExternalInput").ap()
    wvt = nc.dram_tensor("wvt", [D, GD], bf16, kind="ExternalInput").ap()
    wota0 = nc.dram_tensor("wota0", [P, D], bf16, kind="ExternalInput").ap()
    wota1 = nc.dram_tensor("wota1", [65, D], bf16, kind="ExternalInput").ap()
    zeros8 = nc.dram_tensor("zeros8", [P, S], f8, kind="ExternalInput").ap()
    vones = nc.dram_tensor("vones", [P, NKB, GH, 65], bf16,
                           kind="ExternalInput").ap()
    out = nc.dram_tensor("out", [SL, D], f32, kind="ExternalOutput").ap()
    if DRPROJ:
        xt8 = nc.dram_tensor("xt8", [D, S], f8, kind="ExternalInput").ap()
        xtq8 = nc.dram_tensor("xtq8", [D, SL], f8, kind="ExternalInput").ap()
        wq2 = nc.dram_tensor("wq2", [P, NC, 2, GD], f8, kind="ExternalInput").ap()
        wk2 = nc.dram_tensor("wk2", [P, NC, 2, GD], f8, kind="ExternalInput").ap()
    else:
        xtq = nc.dram_tensor("xtq", [D, SL], bf16, kind="ExternalInput").ap()
        wqt = nc.dram_tensor("wqt", [D, GD], bf16, kind="ExternalInput").ap()
        wkt = nc.dram_tensor("wkt", [D, GD], bf16, kind="ExternalInput").ap()

    Exp = mybir.ActivationFunctionType.Exp
    mult = mybir.AluOpType.mult
    add = mybir.AluOpType.add
    DR = mybir.MatmulPerfMode.DoubleRow

    with tile.TileContext(nc) as tc, ExitStack() as ctx:
        const = ctx.enter_context(tc.tile_pool(name="const", bufs=1))

        # ---- persistent SBUF tiles ----
        xt_sb = const.tile([P, NC, S], bf16, tag="xt")
        wvt_sb = const.tile([P, NC, GD], bf16, tag="wvt")
        wota0_sb = const.tile([P, D], bf16, tag="wota0")
        wota1_sb = const.tile([65, D], bf16, tag="wota1")
        bias_sb = const.tile([P, 4], f32, tag="bias")
        mask_sb = const.tile([P, 4, QT], bf16, tag="masks")
        if DRPROJ:
            xt8_sb = const.tile([P, NC, S], f8, tag="xt8")
            xtq8_sb = const.tile([P, NC, SL], f8, tag="xtq8")
            wq2_sb = const.tile([P, NC, 2, GD], f8, tag="wq2")
            wk2_sb = const.tile([P, NC, 2, GD], f8, tag="wk2")
        else:
            xtq_sb = const.tile([P, NC, SL], bf16, tag="xtq")
            wqt_sb = const.tile([P, NC, GD], bf16, tag="wqt")
            wkt_sb = const.tile([P, NC, GD], bf16, tag="wkt")
        # fp8 K cache, 3 heads packed into (partition-half, rail):
        #   (p 0:64,  rail 0) = head0 dims, (p 64:128, rail 0) = head1 dims,
        #   (p 0:64,  rail 1) = head2 dims, (p 64:128, rail 1) = ZERO.
        k8_sb = const.tile([P, 2, S], f8, tag="k8")
        # per-head fp8 Q, same packing; zeros everywhere except own region so
        # the shared K stationary operand only contracts the own head's dims.
        q8_sb = [const.tile([P, 2, SL], f8, tag=f"q8{h}", name=f"q8{h}")
                 for h in range(GH)]
        # V per key block: [128 k-part, kb, head, 65] with col 64 = 1.0
        v_sb = const.tile([P, NKB, GH, 65], bf16, tag="vall")

        # ---- weights on the gpsimd (SWDGE) queue, need-ordered ----
        nc.gpsimd.dma_start(bias_sb[:], bias[:])
        if DRPROJ:
            nc.gpsimd.dma_start(wq2_sb[:], wq2[:])
        else:
            nc.gpsimd.dma_start(wqt_sb[:], wqt.rearrange("(c p) d -> p c d", p=P))
        nc.gpsimd.dma_start(mask_sb[:], masks.rearrange("p (w x) -> p w x", x=QT))
        if DRPROJ:
            nc.gpsimd.dma_start(wk2_sb[:], wk2[:])
        else:
            nc.gpsimd.dma_start(wkt_sb[:], wkt.rearrange("(c p) d -> p c d", p=P))
        nc.gpsimd.dma_start(wvt_sb[:], wvt.rearrange("(c p) d -> p c d", p=P))
        nc.gpsimd.dma_start(wota0_sb[:], wota0[:])
        nc.gpsimd.dma_start(wota1_sb[:], wota1[:])

        # ---- inputs + region-exact zero fills (SP queue, JIT-ordered) ----
        # zero fills cover exactly the q8/k8 regions evictions never touch,
        # so they don't fence the eviction writes.
        xt_r = xt.rearrange("(c p) s -> p c s", p=P)
        if DRPROJ:
            xt8_r = xt8.rearrange("(c p) s -> p c s", p=P)
            xtq8_r = xtq8.rearrange("(c p) s -> p c s", p=P)
        else:
            xtq_r = xtq.rearrange("(c p) s -> p c s", p=P)

        def emit_windows(qg):
            qw = slice(qg * QGS, (qg + 1) * QGS)
            if DRPROJ:
                nc.sync.dma_start(xtq8_sb[:, :, qw], xtq8_r[:, :, qw])
            else:
                nc.sync.dma_start(xtq_sb[:, :, qw], xtq_r[:, :, qw])
            for kg in (2 * qg, 2 * qg + 1):
                kw = slice(kg * QGS, (kg + 1) * QGS)
                if DRPROJ:
                    nc.sync.dma_start(xt8_sb[:, :, kw], xt8_r[:, :, kw])
                nc.sync.dma_start(xt_sb[:, :, kw], xt_r[:, :, kw])

        qw0 = slice(0, QGS)
        if DRPROJ:
            nc.sync.dma_start(xtq8_sb[:, :, qw0], xtq8_r[:, :, qw0])
        else:
            nc.sync.dma_start(xtq_sb[:, :, qw0], xtq_r[:, :, qw0])
        for h in range(GH):
            nc.sync.dma_start(q8_sb[h][:],
                              zeros8.rearrange("p (a b) -> p a b", a=2))
        if DRPROJ:
            nc.sync.dma_start(xt8_sb[:, :, qw0], xt8_r[:, :, qw0])
        nc.sync.dma_start(k8_sb[:, 0, :], zeros8[:, :])
        nc.sync.dma_start(k8_sb[:, 1, :], zeros8[:, :])
        nc.sync.dma_start(xt_sb[:, :, qw0], xt_r[:, :, qw0])
        nc.sync.dma_start(v_sb[:, 0:8], vones[:, 0:8])
        kw1 = slice(QGS, 2 * QGS)
        if DRPROJ:
            nc.sync.dma_start(xt8_sb[:, :, kw1], xt8_r[:, :, kw1])
        nc.sync.dma_start(xt_sb[:, :, kw1], xt_r[:, :, kw1])
        nc.sync.dma_start(v_sb[:, 8:NKB], vones[:, 8:NKB])
        for qg in range(1, QG):
            emit_windows(qg)

        evsc = 1.0 / WSCALE if DRPROJ else 1.0

        def proj_mms(ps, w2_sb, wt_sb, x8_sb, x_sb, cols, mrange):
            """contraction loop for one Q/K projection psum"""
            n = cols.stop - cols.start
            for c in range(NC):
                if DRPROJ:
                    rhs = (x8_sb[:, c, cols].unsqueeze(1)
                           .broadcast_to([P, 2, n]))
                    nc.tensor.matmul(ps[:], w2_sb[:, c, :, mrange], rhs,
                                     start=(c == 0), stop=(c == NC - 1),
                                     perf_mode=DR)
                else:
                    nc.tensor.matmul(ps[:], wt_sb[:, c, mrange],
                                     x_sb[:, c, cols],
                                     start=(c == 0), stop=(c == NC - 1))

        with tc.tile_pool(name="sc_ps", bufs=2, space="PSUM") as scps, \
             tc.tile_pool(name="pv_ps", bufs=3, space="PSUM") as pvps, \
             tc.tile_pool(name="pa_ps", bufs=1, space="PSUM") as paps, \
             tc.tile_pool(name="pt", bufs=8) as ptpool, \
             tc.tile_pool(name="stk", bufs=2) as stkpool, \
             tc.tile_pool(name="nrm", bufs=4) as nrmpool, \
             tc.tile_pool(name="oev", bufs=4) as oevpool:

            def qprojA(qg, use_sc):
                qw = slice(qg * QGS, (qg + 1) * QGS)
                if use_sc:
                    ps = scps.tile([P, 2 * QGS], f32, tag="sc",
                                   name=f"qps{qg}")[:, 0:QGS]
                else:
                    ps = paps.tile([P, QGS], f32, tag="pa", name=f"qps{qg}")[:]
                proj_mms(ps, DRPROJ and wq2_sb, not DRPROJ and wqt_sb,
                         DRPROJ and xtq8_sb, not DRPROJ and xtq_sb,
                         qw, slice(0, 128))
                nc.vector.tensor_scalar(
                    q8_sb[0][0:64, 0, qw], ps[0:64, :],
                    evsc, bias_sb[0:64, 0:1], mult, add)
                nc.vector.tensor_scalar(
                    q8_sb[1][64:128, 0, qw], ps[64:128, :],
                    evsc, bias_sb[64:128, 0:1], mult, add)

            def qprojB(qg, use_sc=False):
                qw = slice(qg * QGS, (qg + 1) * QGS)
                if use_sc:
                    ps2 = scps.tile([P, 2 * QGS], f32, tag="sc",
                                    name=f"qps2_{qg}")[0:64, 0:QGS]
                else:
                    ps2 = paps.tile([64, QGS], f32, tag="pa",
                                    name=f"qps2_{qg}")[:]
                proj_mms(ps2, DRPROJ and wq2_sb, not DRPROJ and wqt_sb,
                         DRPROJ and xtq8_sb, not DRPROJ and xtq_sb,
                         qw, slice(128, 192))
                nc.vector.tensor_scalar(
                    q8_sb[2][0:64, 1, qw], ps2,
                    evsc, bias_sb[0:64, 1:2], mult, add)

            def kprojA(kg, use_sc):
                kcols = slice(kg * QGS, (kg + 1) * QGS)
                if use_sc:
                    ps = scps.tile([P, 2 * QGS], f32, tag="sc",
                                   name=f"kps{kg}")[:, 0:QGS]
                else:
                    ps = paps.tile([P, QGS], f32, tag="pa", name=f"kps{kg}")[:]
                proj_mms(ps, DRPROJ and wk2_sb, not DRPROJ and wkt_sb,
                         DRPROJ and xt8_sb, not DRPROJ and xt_sb,
                         kcols, slice(0, 128))
                nc.vector.tensor_scalar(
                    k8_sb[:, 0, kcols], ps[:],
                    evsc, bias_sb[:, 2:3], mult, add)

            def kprojB(kg, use_sc=False):
                kcols = slice(kg * QGS, (kg + 1) * QGS)
                if use_sc:
                    ps2 = scps.tile([P, 2 * QGS], f32, tag="sc",
                                    name=f"kps2_{kg}")[0:64, 0:QGS]
                else:
                    ps2 = paps.tile([64, QGS], f32, tag="pa",
                                    name=f"kps2_{kg}")[:]
                proj_mms(ps2, DRPROJ and wk2_sb, not DRPROJ and wkt_sb,
                         DRPROJ and xt8_sb, not DRPROJ and xt_sb,
                         kcols, slice(128, 192))
                nc.vector.tensor_scalar(
                    k8_sb[0:64, 1, kcols], ps2,
                    evsc, bias_sb[0:64, 3:4], mult, add)

            def vproj(kb, use_sc=False):
                if use_sc:
                    psv = scps.tile([P, 2 * QGS], f32, tag="sc",
                                    name=f"vps{kb}")[:, 0:GD]
                else:
                    psv = paps.tile([P, GD], f32, tag="pa",
                                    name=f"vps{kb}")[:]
                for c in range(NC):
                    nc.tensor.matmul(
                        psv, xt_sb[:, c, kb * P:(kb + 1) * P],
                        wvt_sb[:, c, :],
                        start=(c == 0), stop=(c == NC - 1))
                nc.vector.tensor_copy(
                    v_sb[:, kb, :, 0:64],
                    psv.rearrange("p (h d) -> p h d", d=HD))

            # -- deadline-paced projection feed --------------------------
            # each entry: (t_deadline, m_deadline, emit_fn); the attention
            # m-loop emits every group whose deadline has arrived.
            feed = []
            alt = [0]

            def sc_ok(tdl):
                return False

            for qg in range(QG):
                tq = max(0, 2 * qg - 1)
                if qg > 0:  # qg0's A-halves run at startup on the score banks
                    feed.append((tq, 0,
                                 lambda qg=qg, s=sc_ok(tq): qprojA(qg, s)))
                feed.append((tq, 0, lambda qg=qg, s=sc_ok(tq): qprojB(qg, s)))
                for kg in (2 * qg, 2 * qg + 1):
                    tk, mk = kg, max(0, 4 * kg - 5)
                    if kg > 0:
                        feed.append((tk, mk,
                                     lambda kg=kg, s=sc_ok(tk): kprojA(kg, s)))
                    feed.append((tk, mk,
                                 lambda kg=kg, s=sc_ok(tk): kprojB(kg, s)))
                for kb in range(8 * qg, 8 * qg + 8):
                    feed.append((kb // 4, max(0, kb - 3),
                                 lambda kb=kb, s=sc_ok(kb // 4): vproj(kb, s)))
            feed.sort(key=lambda e: (e[0], e[1]))

            def run_feed(t, m):
                while feed and (feed[0][0], feed[0][1]) <= (t, m):
                    feed.pop(0)[2]()

            # startup: Q0 / K-kg0-A on the idle score banks, rest from feed
            qprojA(0, True)
            kprojA(0, True)

            def attention(t):
                kcnt = 4 * (t + 1)
                pv = [pvps.tile([65, QGS], f32, tag="pv", name=f"pv{t}_{h}")
                      for h in range(GH)]
                for m in range(kcnt):
                    run_feed(t, m)
                    # causal query-suffix trim: for key block m, local query
                    # tiles j < ceil((m-1)/2) - 2t are entirely below the
                    # diagonal for BOTH parities; data masks handle leftovers.
                    q0 = P * (max(0, -(-(m - 1) // 2) - 2 * t) if m > 0 else 0)
                    sc = scps.tile([P, 2 * QGS], f32, tag="sc",
                                   name=f"sc{t}_{m}")
                    sc_v = sc[:].rearrange("p (h x) -> p h x", x=QT)
                    kblk = k8_sb[:, :, m * P:(m + 1) * P]
                    for h in range(GH):
                        nc.tensor.matmul(
                            sc_v[:, h, q0:], kblk,
                            q8_sb[h][:, :, t * QT + q0:(t + 1) * QT],
                            start=True, stop=True, perf_mode=DR)
                    pt = ptpool.tile([P, GH, QT], bf16, tag="pt")
                    # scores are raw q.k; the 1/sqrt(64) lives in the Exp scale
                    nc.scalar.activation(pt[:, :, q0:], sc_v[:, 0:GH, q0:],
                                         Exp, scale=0.125)
                    if m >= 4 * t:
                        # multiplicative 0/1 causal mask on the probabilities
                        w = m - 4 * t
                        span = P * (w // 2 + 1)
                        if span > q0:
                            for h in range(GH):
                                nc.gpsimd.tensor_tensor(
                                    pt[:, h, q0:span], pt[:, h, q0:span],
                                    mask_sb[:, w, q0:span], mult)
                    for h in range(GH):
                        nc.tensor.matmul(
                            pv[h][:, q0:QT], v_sb[:, m, h, :], pt[:, h, q0:],
                            start=(m == 0), stop=(m == kcnt - 1))
                # normalize: out_h = pv_h[0:64] / pv_h[64] ; stack for out-proj
                stk0 = stkpool.tile([P, QT], bf16, tag="sc0")
                stk1 = stkpool.tile([65, QT], bf16, tag="sc1")
                for h in range(GH):
                    recip = nrmpool.tile([1, QT], bf16, tag="recip")
                    with nc.allow_low_precision(
                            reason="bf16 softmax denominators, ~0.4% rel"):
                        nc.vector.reciprocal(recip[:], pv[h][64:65, 0:QT])
                    bcast = nrmpool.tile([65, QT], bf16, tag="bcast")
                    nc.gpsimd.partition_broadcast(bcast[:], recip[:])
                    if h < 2:
                        nc.vector.tensor_tensor(
                            stk0[h * 64:(h + 1) * 64, :], pv[h][0:64, 0:QT],
                            bcast[0:64, :], mult)
                    else:
                        # rows 0:64 = normalized h2; row 64 = denom/denom ~ 1
                        nc.vector.tensor_tensor(
                            stk1[:], pv[h][0:65, 0:QT], bcast[:], mult)
                # output projection for this group's 2 query tiles; the last
                # group spreads over the freed scores/pv banks and splits the
                # store so copy/DMA overlap.
                last = t == NT - 1
                for jj in range(2):
                    op0 = pvps.tile([P, 384], f32, tag="pv",
                                    name=f"op0_{t}_{jj}")[:]
                    if last:
                        op1 = scps.tile([P, 2 * QGS], f32, tag="sc",
                                        name=f"op1_{t}_{jj}")[:, 0:384]
                    else:
                        op1 = paps.tile([P, 384], f32, tag="pa",
                                        name=f"op1_{t}_{jj}")[:]
                    for half, op in ((0, op0), (1, op1)):
                        nc.tensor.matmul(
                            op, stk0[:, jj * P:(jj + 1) * P],
                            wota0_sb[:, half * 384:(half + 1) * 384],
                            start=True, stop=False)
                        nc.tensor.matmul(
                            op, stk1[:, jj * P:(jj + 1) * P],
                            wota1_sb[:, half * 384:(half + 1) * 384],
                            start=False, stop=True)
                    jq = 2 * t + jj
                    oe = oevpool.tile([P, D], f32, tag="oe")
                    if last:
                        nc.vector.tensor_copy(oe[:, 0:384], op0)
                        nc.sync.dma_start(out[jq * P:(jq + 1) * P, 0:384],
                                          oe[:, 0:384])
                        nc.vector.tensor_copy(oe[:, 384:768], op1)
                        nc.sync.dma_start(out[jq * P:(jq + 1) * P, 384:768],
                                          oe[:, 384:768])
                    else:
                        nc.vector.tensor_copy(oe[:, 0:384], op0)
                        nc.vector.tensor_copy(oe[:, 384:768], op1)
                        nc.sync.dma_start(out[jq * P:(jq + 1) * P, :], oe[:])

            for t in range(NT):
                attention(t)
            # flush any remaining feed entries (shouldn't be any)
            while feed:
                feed.pop(0)[2]()

    nc.compile()
    return nc


def _host_prep(inputs, Wq, bq, Wk, bk, Wv, bv, Wo, bo):
    import ml_dtypes

    bf16 = ml_dtypes.bfloat16
    f8 = ml_dtypes.float8_e4m3
    X = np.asarray(inputs, np.float32).reshape(S, D)
    XT = np.ascontiguousarray(X.T)                      # [768, 4096]
    XT_bf = XT.astype(bf16)
    XT_f8 = XT.astype(f8)
    # query-set gathers: blocks s, s+2, ... of 32 128-col blocks
    XTb = XT.reshape(D, NKB // 2, 2, P)
    XTq = [np.ascontiguousarray(XTb[:, :, s, :].reshape(D, SL)) for s in range(2)]

    # per-core multiplicative causal masks [128, 4, 256], 1=keep 0=drop
    # (S_T layout: k on partitions, q on free dim); w = m - 4t
    tri = (np.arange(P)[None, :] >= np.arange(P)[:, None]).astype(np.float32)
    mk = []
    for s_ in range(2):
        m = np.ones((P, 4, QT), np.float32)
        for w in range(4):
            npref = max(0, -(-(w - s_) // 2))  # ceil((w - s)/2) clamped at 0
            m[:, w, :P * npref] = 0.0
            if w >= s_ and (w - s_) % 2 == 0:
                dblk = (w - s_) // 2
                m[:, w, dblk * P:(dblk + 1) * P] = tri
        mk.append(np.ascontiguousarray(m.reshape(P, 4 * QT)).astype(bf16))

    zeros8 = np.zeros((P, S), f8)
    vones_h = np.ones((P, NKB, GH, 65), bf16)

    def dual_rail(WT):  # [768, 192] f32 -> [128, 6, 2, 192] fp8 (hi, lo)
        Ws = WT * WSCALE
        hi = Ws.astype(f8)
        lo = (Ws - hi.astype(np.float32)).astype(f8)
        w2 = np.zeros((P, NC, 2, GD), np.float32)
        for c in range(NC):
            w2[:, c, 0, :] = hi[c * P:(c + 1) * P].astype(np.float32)
            w2[:, c, 1, :] = lo[c * P:(c + 1) * P].astype(np.float32)
        return np.ascontiguousarray(w2).astype(f8)

    in_maps = []
    for g in range(NG):
        hs = slice(GD * g, GD * (g + 1))
        WqT = np.ascontiguousarray(Wq[hs, :].T).astype(np.float32)
        WkT = np.ascontiguousarray(Wk[hs, :].T).astype(np.float32)
        WvT = np.ascontiguousarray(Wv[hs, :].T).astype(bf16)
        WoT = np.ascontiguousarray(Wo[:, hs].T).astype(np.float32)  # [192, 768]
        bo_g = bv[hs].astype(np.float32) @ WoT
        if g == 0:
            bo_g = bo_g + bo.astype(np.float32)
        wota = np.concatenate([WoT, bo_g[None, :]], axis=0)  # [193, 768]
        wota0 = np.ascontiguousarray(wota[0:P]).astype(bf16)
        wota1 = np.ascontiguousarray(wota[P:]).astype(bf16)
        bias_t = np.zeros((P, 4), np.float32)
        bias_t[:, 0] = bq[hs][0:128]
        bias_t[0:64, 1] = bq[hs][128:192]
        bias_t[:, 2] = bk[hs][0:128]
        bias_t[0:64, 3] = bk[hs][128:192]
        base = {
            "xt": XT_bf, "wvt": WvT,
            "wota0": wota0, "wota1": wota1,
            "bias": bias_t, "zeros8": zeros8, "vones": vones_h,
        }
        if DRPROJ:
            base["xt8"] = XT_f8
            base["wq2"] = dual_rail(WqT)
            base["wk2"] = dual_rail(WkT)
        else:
            base["wqt"] = WqT.astype(bf16)
            base["wkt"] = WkT.astype(bf16)
        for s_ in range(2):
            m = dict(base)
            m["masks"] = mk[s_]
            if DRPROJ:
                m["xtq8"] = XTq[s_].astype(f8)
            else:
                m["xtq"] = XTq[s_].astype(bf16)
            in_maps.append(m)
    return in_maps


def _gather(results):
    out = np.zeros((S, D), np.float32)
    ov = out.reshape(NQB, 2, P, D)
    for s_ in range(2):
        acc = np.zeros((SL, D), np.float32)
        for g in range(NG):
            acc += np.asarray(results[2 * g + s_]["out"], np.float32)
        ov[:, s_, :, :] = acc.reshape(NQB, P, D)
    return out.reshape(1, S, D)


def kernel(inputs, Wq, bq, Wk, bk, Wv, bv, Wo, bo):
    from concourse.bass_utils import run_bass_kernel_spmd

    if "nc" not in _CACHE:
        _CACHE["nc"] = _build_program()
    nc = _CACHE["nc"]
    in_maps = _host_prep(
        np.asarray(inputs), np.asarray(Wq), np.asarray(bq), np.asarray(Wk),
        np.asarray(bk), np.asarray(Wv), np.asarray(bv), np.asarray(Wo),
        np.asarray(bo))
    # core order: core = 2*g + s; in_maps was built g-major with s inner.
    res = run_bass_kernel_spmd(nc, in_maps, list(range(8))).results
    return _gather(res)
